# revision 32
# baseline (speedup 1.0000x reference)
"""Fused multi-head attention (B=2, N=2048, C=1024, H=16) on 8 TRN2 NeuronCores.

Sharding: core = (b, g) with b = batch (2) and g = head-group of 4 heads (4).
Each core computes, for its batch and 4 heads:
    qkv slice -> per-head softmax attention -> out-proj partial (row-parallel).
Host sums the 4 per-head-group proj partials per batch and adds b_proj.

Device algorithm (per core), everything f32 with matmuls in float32r
(full-rate fp32 PE mode) or bf16:
  phase 1: qkT = (x @ Wqk)^T   [512 feats=q4|k4 heads, 2048 tokens]
           v   = x @ Wv        [2048 tokens, 4*64] (+ ones column per head)
  phase 2: per (head, 512-row chunk):
           S^T tiles = kT_h^T-matmul  [128 keys, 512 rows] (K=64, row-tiled
             pairs of heads on PE partitions 0-63 / 64-127)
           expST = exp(S^T/8)  (ScalarE, PSUM->SBUF)
           outT[65, rows] += [v_h|1]^T-matmul expST  (K=128 keys)
             row 64 = softmax denominator (ones column trick)
           outT[0:64] *= 1/denominator  (GpSimd partition_broadcast + DVE)
  phase 3: partial = out^T-matmul Wp   [2048, 1024] -> DMA out
"""

import os
from contextlib import ExitStack

import numpy as np

import concourse.bass as bass
import concourse.mybir as mybir
import concourse.tile as tile
from concourse import bacc
from concourse.bass_utils import run_bass_kernel_spmd

B, N, C = 2, 2048, 1024
HC = 4  # heads per core
D = 64
NCORES = 8
KC = C // 128  # 8 contraction chunks for phase 1
SCALE = D**-0.5  # 0.125

# "f32r" (fp32 data, full-rate PE mode), "bf16", or "f32" (4x slower PE)
MM_DT = os.environ.get("ATTN_MM_DT", "f32r")
ST_TILE_POS = os.environ.get("ATTN_ST_TILE_POS", "1") == "1"
ACT_COPY = os.environ.get("ATTN_ACT_COPY", "0") == "1"


def _np_in_dtype():
    if MM_DT == "bf16":
        import ml_dtypes

        return np.dtype(ml_dtypes.bfloat16)
    return np.dtype(np.float32)


def _prep(a):
    """Cast to the device input dtype; for f32r, pre-round to TF32 (RTNE)."""
    a = np.ascontiguousarray(a)
    if MM_DT != "f32r":
        return a.astype(_np_in_dtype())
    u = a.astype(np.float32).view(np.uint32)
    u = (u + 0x0FFF + ((u >> 13) & 1)) & np.uint32(0xFFFFE000)
    return u.view(np.float32)



def _copy(eng, out, in_):
    if hasattr(eng, "tensor_copy"):
        eng.tensor_copy(out, in_)
    else:
        eng.copy(out, in_)

def build_nc():
    f32 = mybir.dt.float32
    in_dt = {
        "bf16": mybir.dt.bfloat16,
        "f32r": mybir.dt.float32r,
        "f32": mybir.dt.float32,
    }[MM_DT]
    mm = lambda ap: ap  # noqa: E731

    nc = bacc.Bacc("TRN2", target_bir_lowering=False, debug=False, num_devices=NCORES)
    xT_d = nc.dram_tensor("xT", [C, N], in_dt, kind="ExternalInput").ap()
    wqk_d = nc.dram_tensor("wqk", [C, 2 * HC * D], in_dt, kind="ExternalInput").ap()
    wv_d = nc.dram_tensor("wv", [C, HC * D], in_dt, kind="ExternalInput").ap()
    wp_d = nc.dram_tensor("wp", [HC * D, C], in_dt, kind="ExternalInput").ap()
    out_d = nc.dram_tensor("out", [N, C], f32, kind="ExternalOutput").ap()

    with tile.TileContext(nc) as tc:
        with (
            tc.tile_pool(name="const", bufs=1) as const,
            tc.tile_pool(name="ex", bufs=8) as expool,
            tc.tile_pool(name="den", bufs=4) as dpool,
            tc.tile_pool(name="stage", bufs=4) as stage,
            tc.tile_pool(name="stps", bufs=2, space="PSUM") as stps,
            tc.tile_pool(name="pvps", bufs=4, space="PSUM") as pvps,
        ):
            # persistent tiles
            # qkT chunks: 0 = q heads 0,1; 1 = q heads 2,3
            #   (head even -> partitions 0:64, odd -> 64:128)
            # kTp: per-head zero-padded K=128 stationary operand: head even
            #   has kT in rows 0:64 / zeros in 64:128, head odd the reverse,
            #   so a full-128-row matmul against the stacked q chunk
            #   contracts only the matching head's 64 features.
            qkT_sb = const.tile([128, 2, N], in_dt, tag="qkT")
            kTp_sb = const.tile([128, HC, N], in_dt, tag="kTp")
            v_sb = const.tile([128, 16, HC, D + 1], in_dt, tag="v")
            wp_sb = const.tile([128, 2, C], in_dt, tag="wp")
            outT_sb = const.tile([128, 2, N], in_dt, tag="outT")
            xT_sb = const.tile([128, KC, N], in_dt, tag="xT")
            wqk_sb = const.tile([128, KC, 2 * HC * D], in_dt, tag="wqk")
            wv_sb = const.tile([128, KC, HC * D], in_dt, tag="wv")

            # ---- DMAs (wqk+xT first: they gate the first matmuls) ----
            for kc in range(KC):
                nc.sync.dma_start(wqk_sb[:, kc, :], wqk_d[kc * 128 : (kc + 1) * 128, :])
                nc.sync.dma_start(xT_sb[:, kc, :], xT_d[kc * 128 : (kc + 1) * 128, :])
            for kc in range(KC):
                nc.sync.dma_start(wv_sb[:, kc, :], wv_d[kc * 128 : (kc + 1) * 128, :])
            for c2 in range(2):
                nc.sync.dma_start(wp_sb[:, c2, :], wp_d[c2 * 128 : (c2 + 1) * 128, :])

            # ---- one-time fills (run during the DMA wait) ----
            zsrc = const.tile([64, 512], f32, tag="zsrc")
            nc.vector.memset(zsrc[:], 0.0)
            for h in range(HC):
                zb = 64 if h % 2 == 0 else 0
                for nt in range(4):
                    nc.vector.tensor_copy(
                        kTp_sb[zb : zb + 64, h, nt * 512 : (nt + 1) * 512], zsrc[:]
                    )
            ones_f32 = const.tile([128, 16, HC, 1], f32, tag="ones")
            nc.vector.memset(ones_f32[:], 1.0)
            nc.vector.tensor_copy(v_sb[:, :, :, D : D + 1], ones_f32[:])

            # ---- emission helpers ----
            def qk_chunk(mf, nt):
                """One psum of (x @ Wqk)^T: feat chunk mf, token chunk nt.
                wqk feat chunks: 0 = q heads 0,1; 1 = q heads 2,3;
                2 = k heads 0,1; 3 = k heads 2,3."""
                big = mf in (0, 2)  # q01/k01 computed up front on stps
                if big:
                    ps = stps.tile([128, 1024], f32, tag="st", name="st")[:, :512]
                else:
                    ps = pvps.tile([128, 512], f32, tag="pv", name="pv")
                for kc in range(KC):
                    nc.tensor.matmul(
                        ps,
                        mm(wqk_sb[:, kc, mf * 128 : (mf + 1) * 128]),
                        mm(xT_sb[:, kc, nt * 512 : (nt + 1) * 512]),
                        start=(kc == 0),
                        stop=(kc == KC - 1),
                    )
                nts = slice(nt * 512, (nt + 1) * 512)
                if mf % 2 == 0:
                    mq = mf // 2
                    if mf < 2:
                        nc.vector.tensor_copy(qkT_sb[:, mq, nts], ps)
                    else:
                        pass
                if mf < 2:
                    if mf == 1:
                        nc.vector.tensor_copy(qkT_sb[:, 1, nts], ps)
                else:
                    h0, h1 = 2 * (mf - 2), 2 * (mf - 2) + 1
                    nc.vector.tensor_copy(kTp_sb[0:64, h0, nts], ps[0:64, :])
                    nc.vector.tensor_copy(kTp_sb[64:128, h1, nts], ps[64:128, :])

            def v_chunk(t):
                """One psum of v = x @ Wv for token(=key) chunk t, all heads."""
                ps = pvps.tile([128, 512], f32, tag="pv", name="pv")[:, : HC * D]
                for kc in range(KC):
                    nc.tensor.matmul(
                        ps,
                        mm(xT_sb[:, kc, t * 128 : (t + 1) * 128]),
                        mm(wv_sb[:, kc, :]),
                        start=(kc == 0),
                        stop=(kc == KC - 1),
                    )
                nc.vector.tensor_copy(
                    v_sb[:, t, :, 0:D], ps.rearrange("p (h d) -> p h d", h=HC)
                )

            def proj_chunk(t, nf):
                """partial[t*128:(t+1)*128, nf*512:(nf+1)*512] = out @ Wp."""
                ps = pvps.tile([128, 512], f32, tag="pv", name="pv")
                for c2 in range(2):
                    nc.tensor.matmul(
                        ps,
                        mm(outT_sb[:, c2, t * 128 : (t + 1) * 128]),
                        mm(wp_sb[:, c2, nf * 512 : (nf + 1) * 512]),
                        start=(c2 == 0),
                        stop=(c2 == 1),
                    )
                sg = stage.tile([128, 512], f32, tag="sg", name="sg")
                nc.vector.tensor_copy(sg, ps)
                nc.sync.dma_start(
                    out_d[t * 128 : (t + 1) * 128, nf * 512 : (nf + 1) * 512], sg
                )

            # fill queue: work interleaved into attention blocks' PE slack
            fills = []

            def attention_block(hp, rc, sched):
                """ST + exp + PV for head pair hp, 512-row chunk rc; pops
                sched[kc2] fill closures at the top of each kc2 step."""
                heads = (2 * hp, 2 * hp + 1)
                pv = {
                    h: pvps.tile([128, 512], f32, tag="pv", name="pv") for h in heads
                }
                for kc2 in range(8):
                    for _ in range(sched[kc2]):
                        if fills:
                            fills.pop(0)()
                    stp = {
                        h: stps.tile([128, 1024], f32, tag="st", name="st")
                        for h in heads
                    }
                    for j in range(2):
                        kc = 2 * kc2 + j
                        for h in heads:
                            nc.tensor.matmul(
                                stp[h][:, j * 512 : (j + 1) * 512],
                                mm(kTp_sb[:, h, kc * 128 : (kc + 1) * 128]),
                                mm(qkT_sb[:, hp, rc * 512 : (rc + 1) * 512]),
                                start=True,
                                stop=True,
                            )
                    for h in heads:
                        ex = expool.tile([128, 1024], in_dt, tag="ex", name="ex")
                        nc.scalar.activation(
                            ex, stp[h], mybir.ActivationFunctionType.Exp, scale=SCALE
                        )
                        for j in range(2):
                            kc = 2 * kc2 + j
                            nc.tensor.matmul(
                                pv[h][: D + 1, :],
                                mm(v_sb[:, kc, h, :]),
                                mm(ex[:, j * 512 : (j + 1) * 512]),
                                start=(kc == 0),
                                stop=(kc == 15),
                            )
                for h in heads:
                    # evict raw (unnormalized outT + den row) immediately so
                    # the pv PSUM slot frees in ~0.7us; normalize SBUF-side,
                    # fully off the PSUM critical path.
                    raw = dpool.tile([D + 1, 512], f32, tag="raw", name="raw")
                    nc.vector.tensor_copy(raw, pv[h][: D + 1, :])
                    den = dpool.tile([1, 512], f32, tag="den", name="den")
                    nc.vector.reciprocal(den, raw[D : D + 1, :])
                    rbc = dpool.tile([64, 512], f32, tag="rbc", name="rbc")
                    nc.gpsimd.partition_broadcast(rbc, den)
                    hb = (h % 2) * 64
                    nc.vector.tensor_tensor(
                        out=outT_sb[hb : hb + 64, hp, rc * 512 : (rc + 1) * 512],
                        in0=raw[0:D, :],
                        in1=rbc[:],
                        op=mybir.AluOpType.mult,
                    )

            # ---- schedule ----
            # up-front: q and k for heads 0,1 (the minimum to start attention)
            for mf in (0, 2):
                for nt in range(4):
                    qk_chunk(mf, nt)
            # v chunks interleave into block (0,0): v(kc) lands just before
            # its PV(kc) consumer; q23/k23 chunks spread over blocks (0,1..3);
            # proj(rc) spreads into the block after (1,rc).
            def queue_proj(rc):
                fills.extend(
                    [
                        lambda t=t, nf=nf: proj_chunk(t, nf)
                        for t in range(4 * rc, 4 * rc + 4)
                        for nf in range(2)
                    ]
                )

            # Alternating hp order: each (1,rc) block's denominator chain gets
            # a full unrelated block of latency before proj(rc) pops consume
            # its outT, so the PE never stalls on the recip/bcast/mult chain.
            late = [0, 0, 2, 2, 2, 2, 0, 0]
            fills.extend([lambda t=t: v_chunk(t) for t in range(16)])
            attention_block(0, 0, sched=[2] * 8)
            fills.extend([lambda nt=nt: qk_chunk(1, nt) for nt in range(4)])
            fills.extend([lambda nt=nt: qk_chunk(3, nt) for nt in range(4)])
            attention_block(0, 1, sched=[0, 0, 2, 2, 2, 2, 0, 0])
            attention_block(1, 0, sched=[0] * 8)
            queue_proj(0)
            attention_block(0, 2, sched=late)
            attention_block(1, 1, sched=[0] * 8)
            queue_proj(1)
            attention_block(0, 3, sched=late)
            attention_block(1, 2, sched=[0] * 8)
            queue_proj(2)
            attention_block(1, 3, sched=late)
            queue_proj(3)
            # drain any remaining fill work (last proj block)
            while fills:
                fills.pop(0)()
    nc.compile()
    return nc


def make_in_maps(x, w_qkv, w_proj):
    in_maps = []
    for core in range(NCORES):
        b, g = core // 4, core % 4
        qs = slice(g * 256, (g + 1) * 256)
        in_maps.append(
            {
                "xT": _prep(x[b].T),
                "wqk": _prep(
                    np.concatenate(
                        [w_qkv[:, qs], w_qkv[:, C + g * 256 : C + (g + 1) * 256]],
                        axis=1,
                    )
                ),
                "wv": _prep(w_qkv[:, 2 * C + g * 256 : 2 * C + (g + 1) * 256]),
                "wp": _prep(w_proj[qs, :]),
            }
        )
    return in_maps


def run_hw(x, w_qkv, w_proj, b_proj, trace=False):
    """Returns (full output [2, 2048, 1024] f32, exec_time_ns or None)."""
    in_maps = make_in_maps(x, w_qkv, w_proj)
    nc = build_nc()
    r = run_bass_kernel_spmd(nc, in_maps, core_ids=list(range(NCORES)), trace=trace)
    full = np.zeros((B, N, C), np.float32)
    for core in range(NCORES):
        full[core // 4] += r.results[core]["out"]
    full += np.asarray(b_proj, np.float32)[None, None, :]
    return full, r.exec_time_ns


def kernel(**inputs):
    x = np.asarray(inputs["x"], np.float32)
    w_qkv = np.asarray(inputs["w_qkv"], np.float32)
    w_proj = np.asarray(inputs["w_proj"], np.float32)
    b_proj = np.asarray(inputs["b_proj"], np.float32)
    out, _ = run_hw(x, w_qkv, w_proj, b_proj, trace=False)
    return out


# revision 33
# speedup vs baseline: 1.0666x; 1.0666x over previous
"""Fused multi-head attention (B=2, N=2048, C=1024, H=16) on 8 TRN2 NeuronCores.

Sharding: core = (b, g) with b = batch (2) and g = head-group of 4 heads (4).
Each core computes, for its batch and 4 heads:
    qkv slice -> per-head softmax attention -> out-proj partial (row-parallel).
Host sums the 4 per-head-group proj partials per batch and adds b_proj.

Device algorithm (per core), everything f32 with matmuls in float32r
(full-rate fp32 PE mode) or bf16:
  phase 1: qkT = (x @ Wqk)^T   [512 feats=q4|k4 heads, 2048 tokens]
           v   = x @ Wv        [2048 tokens, 4*64] (+ ones column per head)
  phase 2: per (head, 512-row chunk):
           S^T tiles = kT_h^T-matmul  [128 keys, 512 rows] (K=64, row-tiled
             pairs of heads on PE partitions 0-63 / 64-127)
           expST = exp(S^T/8)  (ScalarE, PSUM->SBUF)
           outT[65, rows] += [v_h|1]^T-matmul expST  (K=128 keys)
             row 64 = softmax denominator (ones column trick)
           outT[0:64] *= 1/denominator  (GpSimd partition_broadcast + DVE)
  phase 3: partial = out^T-matmul Wp   [2048, 1024] -> DMA out
"""

import os
from contextlib import ExitStack

import numpy as np

import concourse.bass as bass
import concourse.mybir as mybir
import concourse.tile as tile
from concourse import bacc
from concourse.bass_utils import run_bass_kernel_spmd

B, N, C = 2, 2048, 1024
HC = 4  # heads per core
D = 64
NCORES = 8
KC = C // 128  # 8 contraction chunks for phase 1
SCALE = D**-0.5  # 0.125

# "f32r" (fp32 data, full-rate PE mode), "bf16", or "f32" (4x slower PE)
MM_DT = os.environ.get("ATTN_MM_DT", "bf16")
ST_TILE_POS = os.environ.get("ATTN_ST_TILE_POS", "1") == "1"
ACT_COPY = os.environ.get("ATTN_ACT_COPY", "0") == "1"


def _np_in_dtype():
    if MM_DT == "bf16":
        import ml_dtypes

        return np.dtype(ml_dtypes.bfloat16)
    return np.dtype(np.float32)


def _prep(a):
    """Cast to the device input dtype; for f32r, pre-round to TF32 (RTNE)."""
    a = np.ascontiguousarray(a)
    if MM_DT != "f32r":
        return a.astype(_np_in_dtype())
    u = a.astype(np.float32).view(np.uint32)
    u = (u + 0x0FFF + ((u >> 13) & 1)) & np.uint32(0xFFFFE000)
    return u.view(np.float32)



def _copy(eng, out, in_):
    if hasattr(eng, "tensor_copy"):
        eng.tensor_copy(out, in_)
    else:
        eng.copy(out, in_)

def build_nc():
    f32 = mybir.dt.float32
    in_dt = {
        "bf16": mybir.dt.bfloat16,
        "f32r": mybir.dt.float32r,
        "f32": mybir.dt.float32,
    }[MM_DT]
    mm = lambda ap: ap  # noqa: E731

    nc = bacc.Bacc("TRN2", target_bir_lowering=False, debug=False, num_devices=NCORES)
    xT_d = nc.dram_tensor("xT", [C, N], in_dt, kind="ExternalInput").ap()
    wqk_d = nc.dram_tensor("wqk", [C, 2 * HC * D], in_dt, kind="ExternalInput").ap()
    wv_d = nc.dram_tensor("wv", [C, HC * D], in_dt, kind="ExternalInput").ap()
    wp_d = nc.dram_tensor("wp", [HC * D, C], in_dt, kind="ExternalInput").ap()
    out_d = nc.dram_tensor("out", [N, C], f32, kind="ExternalOutput").ap()

    with tile.TileContext(nc) as tc:
        with (
            tc.tile_pool(name="const", bufs=1) as const,
            tc.tile_pool(name="ex", bufs=8) as expool,
            tc.tile_pool(name="den", bufs=4) as dpool,
            tc.tile_pool(name="stage", bufs=4) as stage,
            tc.tile_pool(name="stps", bufs=2, space="PSUM") as stps,
            tc.tile_pool(name="pvps", bufs=4, space="PSUM") as pvps,
        ):
            # persistent tiles
            # qkT chunks: 0 = q heads 0,1; 1 = q heads 2,3
            #   (head even -> partitions 0:64, odd -> 64:128)
            # kTp: per-head zero-padded K=128 stationary operand: head even
            #   has kT in rows 0:64 / zeros in 64:128, head odd the reverse,
            #   so a full-128-row matmul against the stacked q chunk
            #   contracts only the matching head's 64 features.
            qkT_sb = const.tile([128, 2, N], in_dt, tag="qkT")
            kTp_sb = const.tile([128, HC, N], in_dt, tag="kTp")
            v_sb = const.tile([128, 16, HC, D + 1], in_dt, tag="v")
            wp_sb = const.tile([128, 2, C], in_dt, tag="wp")
            outT_sb = const.tile([128, 2, N], in_dt, tag="outT")
            xT_sb = const.tile([128, KC, N], in_dt, tag="xT")
            wqk_sb = const.tile([128, KC, 2 * HC * D], in_dt, tag="wqk")
            wv_sb = const.tile([128, KC, HC * D], in_dt, tag="wv")

            # ---- DMAs (wqk+xT first: they gate the first matmuls) ----
            for kc in range(KC):
                nc.sync.dma_start(wqk_sb[:, kc, :], wqk_d[kc * 128 : (kc + 1) * 128, :])
                nc.sync.dma_start(xT_sb[:, kc, :], xT_d[kc * 128 : (kc + 1) * 128, :])
            for kc in range(KC):
                nc.sync.dma_start(wv_sb[:, kc, :], wv_d[kc * 128 : (kc + 1) * 128, :])
            for c2 in range(2):
                nc.sync.dma_start(wp_sb[:, c2, :], wp_d[c2 * 128 : (c2 + 1) * 128, :])

            # ---- one-time fills (run during the DMA wait) ----
            zsrc = const.tile([64, 512], f32, tag="zsrc")
            nc.vector.memset(zsrc[:], 0.0)
            for h in range(HC):
                zb = 64 if h % 2 == 0 else 0
                for nt in range(4):
                    nc.vector.tensor_copy(
                        kTp_sb[zb : zb + 64, h, nt * 512 : (nt + 1) * 512], zsrc[:]
                    )
            ones_f32 = const.tile([128, 16, HC, 1], f32, tag="ones")
            nc.vector.memset(ones_f32[:], 1.0)
            nc.vector.tensor_copy(v_sb[:, :, :, D : D + 1], ones_f32[:])

            # ---- emission helpers ----
            def qk_chunk(mf, nt):
                """One psum of (x @ Wqk)^T: feat chunk mf, token chunk nt.
                wqk feat chunks: 0 = q heads 0,1; 1 = q heads 2,3;
                2 = k heads 0,1; 3 = k heads 2,3."""
                big = mf in (0, 2)  # q01/k01 computed up front on stps
                if big:
                    ps = stps.tile([128, 1024], f32, tag="st", name="st")[:, :512]
                else:
                    ps = pvps.tile([128, 512], f32, tag="pv", name="pv")
                for kc in range(KC):
                    nc.tensor.matmul(
                        ps,
                        mm(wqk_sb[:, kc, mf * 128 : (mf + 1) * 128]),
                        mm(xT_sb[:, kc, nt * 512 : (nt + 1) * 512]),
                        start=(kc == 0),
                        stop=(kc == KC - 1),
                    )
                nts = slice(nt * 512, (nt + 1) * 512)
                if mf % 2 == 0:
                    mq = mf // 2
                    if mf < 2:
                        nc.vector.tensor_copy(qkT_sb[:, mq, nts], ps)
                    else:
                        pass
                if mf < 2:
                    if mf == 1:
                        nc.vector.tensor_copy(qkT_sb[:, 1, nts], ps)
                else:
                    h0, h1 = 2 * (mf - 2), 2 * (mf - 2) + 1
                    nc.vector.tensor_copy(kTp_sb[0:64, h0, nts], ps[0:64, :])
                    nc.vector.tensor_copy(kTp_sb[64:128, h1, nts], ps[64:128, :])

            def v_chunk(t):
                """One psum of v = x @ Wv for token(=key) chunk t, all heads."""
                ps = pvps.tile([128, 512], f32, tag="pv", name="pv")[:, : HC * D]
                for kc in range(KC):
                    nc.tensor.matmul(
                        ps,
                        mm(xT_sb[:, kc, t * 128 : (t + 1) * 128]),
                        mm(wv_sb[:, kc, :]),
                        start=(kc == 0),
                        stop=(kc == KC - 1),
                    )
                nc.vector.tensor_copy(
                    v_sb[:, t, :, 0:D], ps.rearrange("p (h d) -> p h d", h=HC)
                )

            def proj_chunk(t, nf):
                """partial[t*128:(t+1)*128, nf*512:(nf+1)*512] = out @ Wp."""
                ps = pvps.tile([128, 512], f32, tag="pv", name="pv")
                for c2 in range(2):
                    nc.tensor.matmul(
                        ps,
                        mm(outT_sb[:, c2, t * 128 : (t + 1) * 128]),
                        mm(wp_sb[:, c2, nf * 512 : (nf + 1) * 512]),
                        start=(c2 == 0),
                        stop=(c2 == 1),
                    )
                sg = stage.tile([128, 512], f32, tag="sg", name="sg")
                nc.vector.tensor_copy(sg, ps)
                nc.sync.dma_start(
                    out_d[t * 128 : (t + 1) * 128, nf * 512 : (nf + 1) * 512], sg
                )

            # fill queue: work interleaved into attention blocks' PE slack
            fills = []

            def attention_block(hp, rc, sched):
                """ST + exp + PV for head pair hp, 512-row chunk rc; pops
                sched[kc2] fill closures at the top of each kc2 step."""
                heads = (2 * hp, 2 * hp + 1)
                pv = {
                    h: pvps.tile([128, 512], f32, tag="pv", name="pv") for h in heads
                }
                for kc2 in range(8):
                    for _ in range(sched[kc2]):
                        if fills:
                            fills.pop(0)()
                    stp = {
                        h: stps.tile([128, 1024], f32, tag="st", name="st")
                        for h in heads
                    }
                    for j in range(2):
                        kc = 2 * kc2 + j
                        for h in heads:
                            nc.tensor.matmul(
                                stp[h][:, j * 512 : (j + 1) * 512],
                                mm(kTp_sb[:, h, kc * 128 : (kc + 1) * 128]),
                                mm(qkT_sb[:, hp, rc * 512 : (rc + 1) * 512]),
                                start=True,
                                stop=True,
                            )
                    for h in heads:
                        ex = expool.tile([128, 1024], in_dt, tag="ex", name="ex")
                        nc.scalar.activation(
                            ex, stp[h], mybir.ActivationFunctionType.Exp, scale=SCALE
                        )
                        for j in range(2):
                            kc = 2 * kc2 + j
                            nc.tensor.matmul(
                                pv[h][: D + 1, :],
                                mm(v_sb[:, kc, h, :]),
                                mm(ex[:, j * 512 : (j + 1) * 512]),
                                start=(kc == 0),
                                stop=(kc == 15),
                            )
                for h in heads:
                    den = dpool.tile([1, 512], f32, tag="den", name="den")
                    nc.vector.reciprocal(den, pv[h][D : D + 1, :])
                    rbc = dpool.tile([64, 512], f32, tag="rbc", name="rbc")
                    nc.gpsimd.partition_broadcast(rbc, den)
                    hb = (h % 2) * 64
                    nc.vector.tensor_tensor(
                        out=outT_sb[hb : hb + 64, hp, rc * 512 : (rc + 1) * 512],
                        in0=pv[h][0:D, :],
                        in1=rbc[:],
                        op=mybir.AluOpType.mult,
                    )

            # ---- schedule ----
            # up-front: q and k for heads 0,1 (the minimum to start attention)
            for mf in (0, 2):
                for nt in range(4):
                    qk_chunk(mf, nt)
            # v chunks interleave into block (0,0): v(kc) lands just before
            # its PV(kc) consumer; q23/k23 chunks spread over blocks (0,1..3);
            # proj(rc) spreads into the block after (1,rc).
            def queue_proj(rc):
                fills.extend(
                    [
                        lambda t=t, nf=nf: proj_chunk(t, nf)
                        for t in range(4 * rc, 4 * rc + 4)
                        for nf in range(2)
                    ]
                )

            # Alternating hp order: each (1,rc) block's denominator chain gets
            # a full unrelated block of latency before proj(rc) pops consume
            # its outT, so the PE never stalls on the recip/bcast/mult chain.
            late = [0, 0, 0, 0, 2, 2, 2, 2]
            fills.extend([lambda t=t: v_chunk(t) for t in range(16)])
            attention_block(0, 0, sched=[2] * 8)
            fills.extend([lambda nt=nt: qk_chunk(1, nt) for nt in range(4)])
            fills.extend([lambda nt=nt: qk_chunk(3, nt) for nt in range(4)])
            attention_block(0, 1, sched=[1] * 8)
            attention_block(1, 0, sched=[0] * 8)
            queue_proj(0)
            attention_block(0, 2, sched=late)
            attention_block(1, 1, sched=[0] * 8)
            queue_proj(1)
            attention_block(0, 3, sched=late)
            attention_block(1, 2, sched=[0] * 8)
            queue_proj(2)
            attention_block(1, 3, sched=late)
            queue_proj(3)
            # drain any remaining fill work (last proj block)
            while fills:
                fills.pop(0)()
    nc.compile()
    return nc


def make_in_maps(x, w_qkv, w_proj):
    in_maps = []
    for core in range(NCORES):
        b, g = core // 4, core % 4
        qs = slice(g * 256, (g + 1) * 256)
        in_maps.append(
            {
                "xT": _prep(x[b].T),
                "wqk": _prep(
                    np.concatenate(
                        [w_qkv[:, qs], w_qkv[:, C + g * 256 : C + (g + 1) * 256]],
                        axis=1,
                    )
                ),
                "wv": _prep(w_qkv[:, 2 * C + g * 256 : 2 * C + (g + 1) * 256]),
                "wp": _prep(w_proj[qs, :]),
            }
        )
    return in_maps


def run_hw(x, w_qkv, w_proj, b_proj, trace=False):
    """Returns (full output [2, 2048, 1024] f32, exec_time_ns or None)."""
    in_maps = make_in_maps(x, w_qkv, w_proj)
    nc = build_nc()
    r = run_bass_kernel_spmd(nc, in_maps, core_ids=list(range(NCORES)), trace=trace)
    full = np.zeros((B, N, C), np.float32)
    for core in range(NCORES):
        full[core // 4] += r.results[core]["out"]
    full += np.asarray(b_proj, np.float32)[None, None, :]
    return full, r.exec_time_ns


def kernel(**inputs):
    x = np.asarray(inputs["x"], np.float32)
    w_qkv = np.asarray(inputs["w_qkv"], np.float32)
    w_proj = np.asarray(inputs["w_proj"], np.float32)
    b_proj = np.asarray(inputs["b_proj"], np.float32)
    out, _ = run_hw(x, w_qkv, w_proj, b_proj, trace=False)
    return out
